# revision 9
# baseline (speedup 1.0000x reference)
"""Trainium2 8-core attention kernel for nn_Attention_14104672600564.

Problem: x[4,128,64,64] f32; wq/wk/wv/wo [128,128]; bo[128].
  per (b,h): sim = (wq x)^T (wk x) * d^-.5 ; attn = softmax(sim) ; out_h = attn @ (wv x)^T
  out = wo @ concat_h(out_h^T) + bo

Sharding: 16 independent (batch, head) attention problems -> 8 cores, each core
gets one batch and one head-PAIR (2 heads). Each core computes its partial
output wo[:, headslice] @ heads_out  [128, 4096]; host unshards by summing the
two cores of each batch.

Per-core pipeline (matmuls bf16, accum f32):
  - project Q,K (scale folded into wq on host) -> SBUF qk[64, 8192]
    (partitions 0-31 head0 / 32-63 head1; free 0-4095 Q, 4096-8191 K)
  - project V^T -> vt[128, 66*32] with a fused ones-column per j-block
    (block layout [1 | V_h0(32) | 1 | V_h1(32)]) so the AV matmul also
    produces the softmax denominator at output partition 0.
  - per head, per i-chunk(512), per j-group(2 strips of 128):
      sim^T tile [128,1024] = K_strip^T Q   (PE)
      P^T = exp(sim^T)                      (ACT, FD=1024, PSUM->SBUF bf16;
                                             no max-subtract: |sim| < ~0.5)
      av[33,512] += [1|V^T]^T P^T           (PE, PSUM accumulate)
  - epilogue per (head, i-chunk): evac av -> avu (DVE), reciprocal of the
    denominator row (DVE), broadcast recip across partitions via a K=1 fp32
    matmul (PE), normalize (DVE), then final projection woT33 (row0=0 kills
    the denominator row) + bias via tensor_scalar_add, DMA out.
"""

import sys

sys.path.insert(0, "/opt/trn_rl_repo")

import numpy as np
import ml_dtypes

import concourse.bass as bass
import concourse.bacc as bacc
import concourse.tile as tile
from concourse import mybir
from concourse.bass_utils import run_bass_kernel_spmd

BF16 = mybir.dt.bfloat16
F32 = mybir.dt.float32

HEADS = 4
DH = 32  # dim per head
C = 128  # channels
N = 4096  # tokens (64*64)
IC = 512  # i-chunk
NICH = N // IC  # 8
JS = 128  # j-strip
NJS = N // JS  # 32
VBLK = 2 * (DH + 1)  # 66: [1|Vh0|1|Vh1] per j-strip

_last_results = None  # test harness pokes this for exec_time_ns / profile


def _build():
    nc = bacc.Bacc(None, target_bir_lowering=False)
    xt_d = nc.declare_dram_parameter("xt", [C, N], BF16, isOutput=False)
    wqkT_d = nc.declare_dram_parameter("wqkT", [C, 128], BF16, isOutput=False)
    wvT_d = nc.declare_dram_parameter("wvT", [C, VBLK], BF16, isOutput=False)
    woT_d = nc.declare_dram_parameter("woT", [DH + 1, 256], BF16, isOutput=False)
    bo_d = nc.declare_dram_parameter("bo", [C, 1], F32, isOutput=False)
    out_d = nc.declare_dram_parameter("out", [C, N], F32, isOutput=True)

    EXP = mybir.ActivationFunctionType.Exp

    with tile.TileContext(nc) as tc:
        with (
            tc.tile_pool(name="singles", bufs=1) as singles,
            tc.tile_pool(name="pts", bufs=3) as pts,
            tc.tile_pool(name="simpool", bufs=2, space="PSUM") as simpool,
            tc.tile_pool(name="avpool", bufs=2, space="PSUM") as avpool,
            tc.tile_pool(name="pbpool", bufs=1, space="PSUM") as pbpool,
            tc.tile_pool(name="popool", bufs=1, space="PSUM") as popool,
        ):
            xt_s = singles.tile([C, N], BF16)
            wqkT_s = singles.tile([C, 128], BF16)
            wvT_s = singles.tile([C, VBLK], BF16)
            woT_s = singles.tile([DH + 1, 256], BF16)
            bo_s = singles.tile([C, 1], F32)
            qk = singles.tile([64, 2 * N], BF16)
            vt = singles.tile([C, VBLK * NJS], BF16)
            avu = singles.tile([DH + 1, 2 * N], F32)
            avn = singles.tile([DH + 1, 2 * N], BF16)
            rec = singles.tile([1, 2 * N], F32)
            ones33 = singles.tile([1, DH + 1], F32)
            outs = singles.tile([C, N], F32)

            nc.sync.dma_start(out=wqkT_s[:], in_=wqkT_d[:])
            nc.sync.dma_start(out=wvT_s[:], in_=wvT_d[:])
            nc.sync.dma_start(out=woT_s[:], in_=woT_d[:])
            nc.sync.dma_start(out=bo_s[:], in_=bo_d[:])
            nc.vector.memset(ones33[:], 1.0)
            for ic in range(NICH):
                nc.sync.dma_start(
                    out=xt_s[:, IC * ic : IC * (ic + 1)],
                    in_=xt_d[:, IC * ic : IC * (ic + 1)],
                )

            # ---- QK projection: out parts 0-63 = [Qh0;Qh1] / [Kh0;Kh1]
            for ic in range(NICH):
                ps = simpool.tile([128, 1024], F32, tag="sim")
                nc.tensor.matmul(
                    ps[0:64, 0:IC],
                    lhsT=wqkT_s[:, 0:64],
                    rhs=xt_s[:, IC * ic : IC * (ic + 1)],
                    start=True,
                    stop=True,
                )
                nc.tensor.matmul(
                    ps[0:64, IC : 2 * IC],
                    lhsT=wqkT_s[:, 64:128],
                    rhs=xt_s[:, IC * ic : IC * (ic + 1)],
                    start=True,
                    stop=True,
                )
                nc.vector.tensor_copy(qk[:, IC * ic : IC * (ic + 1)], ps[0:64, 0:IC])
                nc.vector.tensor_copy(
                    qk[:, N + IC * ic : N + IC * (ic + 1)], ps[0:64, IC : 2 * IC]
                )

            # ---- V^T projection into the ones-laced layout.
            # wvT is host-padded to 66 cols with zeros at cols 0 and 33; the
            # projection writes [junk|Vh0|junk|Vh1] to PSUM, DVE memsets the
            # two junk columns to 1.0 (same-engine, no extra semaphore on the
            # copy), then one contiguous aligned copy lands the whole block.
            for jc in range(NJS):
                pv = simpool.tile([128, 1024], F32, tag="sim")
                nc.tensor.matmul(
                    pv[:, 0:VBLK],
                    lhsT=xt_s[:, JS * jc : JS * (jc + 1)],
                    rhs=wvT_s[:],
                    start=True,
                    stop=True,
                )
                nc.vector.memset(pv[:, 0:1], 1.0)
                nc.vector.memset(pv[:, 33:34], 1.0)
                base = VBLK * jc
                nc.vector.tensor_copy(vt[:, base : base + VBLK], pv[:, 0:VBLK])

            # ---- main attention loops (heads sequential)
            for h in range(2):
                hoff = N * h  # free offset into avu/avn/rec for this head
                for ic in range(NICH):
                    av = avpool.tile([DH + 1, IC], F32, tag="av")
                    for g in range(NJS // 2):
                        sim = simpool.tile([128, 1024], F32, tag="sim")
                        for t in range(2):
                            js = 2 * g + t
                            nc.tensor.matmul(
                                sim[:, IC * t : IC * (t + 1)],
                                lhsT=qk[
                                    32 * h : 32 * h + 32,
                                    N + JS * js : N + JS * (js + 1),
                                ],
                                rhs=qk[32 * h : 32 * h + 32, IC * ic : IC * (ic + 1)],
                                start=True,
                                stop=True,
                            )
                        pt = pts.tile([128, 1024], BF16, tag="pt")
                        nc.scalar.activation(pt[:], sim[:], EXP)
                        for t in range(2):
                            js = 2 * g + t
                            nc.tensor.matmul(
                                av[:],
                                lhsT=vt[
                                    :, VBLK * js + 33 * h : VBLK * js + 33 * h + 33
                                ],
                                rhs=pt[:, IC * t : IC * (t + 1)],
                                start=(g == 0 and t == 0),
                                stop=(g == NJS // 2 - 1 and t == 1),
                            )
                    # ---- per-(head, i-chunk) epilogue
                    sl = slice(hoff + IC * ic, hoff + IC * (ic + 1))
                    nc.vector.tensor_copy(avu[:, sl], av[:])
                    nc.vector.reciprocal(rec[0:1, sl], avu[0:1, sl])
                    pb = pbpool.tile([DH + 1, IC], F32, tag="pb")
                    nc.tensor.matmul(
                        pb[:], lhsT=ones33[:], rhs=rec[0:1, sl], start=True, stop=True
                    )
                    nc.vector.tensor_mul(avn[:, sl], avu[:, sl], pb[:])
                    if h == 1:
                        po = popool.tile([C, IC], F32, tag="po")
                        nc.tensor.matmul(
                            po[:],
                            lhsT=woT_s[:, 0:128],
                            rhs=avn[:, IC * ic : IC * (ic + 1)],
                            start=True,
                            stop=False,
                        )
                        nc.tensor.matmul(
                            po[:],
                            lhsT=woT_s[:, 128:256],
                            rhs=avn[:, N + IC * ic : N + IC * (ic + 1)],
                            start=False,
                            stop=True,
                        )
                        nc.vector.tensor_scalar_add(
                            outs[:, IC * ic : IC * (ic + 1)], po[:], bo_s[:]
                        )
                        nc.sync.dma_start(
                            out=out_d[:, IC * ic : IC * (ic + 1)],
                            in_=outs[:, IC * ic : IC * (ic + 1)],
                        )
    nc.finalize()
    return nc


_nc_cache = None


def _get_nc():
    global _nc_cache
    if _nc_cache is None:
        _nc_cache = _build()
    return _nc_cache


def kernel(x, wq, wk, wv, wo, bo):
    global _last_results
    b = 4
    xt = np.asarray(x, np.float32).reshape(b, C, N)
    wq = np.asarray(wq, np.float32)
    wk = np.asarray(wk, np.float32)
    wv = np.asarray(wv, np.float32)
    wo = np.asarray(wo, np.float32)
    bo = np.asarray(bo, np.float32)
    scale = DH ** (-0.5)

    def bf(a):
        return np.ascontiguousarray(a.astype(ml_dtypes.bfloat16))

    in_maps = []
    for core in range(8):
        bi, hp = core // 2, core % 2
        wq2 = wq[64 * hp : 64 * hp + 64] * scale
        wk2 = wk[64 * hp : 64 * hp + 64]
        wv2 = wv[64 * hp : 64 * hp + 64]
        wqkT = np.concatenate([wq2.T, wk2.T], axis=1)  # [128,128]
        wvT = np.zeros((C, VBLK), np.float32)  # cols 0,33 stay 0 (psum memset->1)
        wvT[:, 1:33] = wv2.T[:, 0:32]
        wvT[:, 34:66] = wv2.T[:, 32:64]
        woT = np.zeros((DH + 1, 256), np.float32)
        woT[1:, 0:128] = wo[:, 64 * hp : 64 * hp + 32].T
        woT[1:, 128:256] = wo[:, 64 * hp + 32 : 64 * hp + 64].T
        bo_in = (bo if hp == 0 else np.zeros_like(bo)).reshape(C, 1).astype(np.float32)
        in_maps.append(
            {
                "xt": bf(xt[bi]),
                "wqkT": bf(wqkT),
                "wvT": bf(wvT),
                "woT": bf(woT),
                "bo": bo_in,
            }
        )

    nc = _get_nc()
    res = run_bass_kernel_spmd(nc, in_maps, core_ids=list(range(8)))
    _last_results = res
    outs = res.results
    out = np.zeros((b, C, N), np.float32)
    for bi in range(b):
        out[bi] = np.asarray(outs[2 * bi]["out"], np.float32) + np.asarray(
            outs[2 * bi + 1]["out"], np.float32
        )
    return out.reshape(b, C, 64, 64)


# revision 10
# speedup vs baseline: 1.2843x; 1.2843x over previous
"""Trainium2 8-core attention kernel for nn_Attention_14104672600564.

Problem: x[4,128,64,64] f32; wq/wk/wv/wo [128,128]; bo[128].
  per (b,h): sim = (wq x)^T (wk x) * d^-.5 ; attn = softmax(sim) ; out_h = attn @ (wv x)^T
  out = wo @ concat_h(out_h^T) + bo

Sharding: 16 independent (batch, head) attention problems -> 8 cores; each core
gets one batch and one head-pair. Each core computes its partial output
wo[:, headslice] @ heads_out [128, 4096]; the host unshards by summing the two
cores of each batch.

Perf design notes:
  - All matmuls bf16 (1 cyc/row; fp32 would be 4), accumulation fp32 in PSUM.
  - EVERY matmul is padded to untiled (128,128) PE mode (K and M padded to >64
    with zero rows/cols). Mixing PE tile modes forces a drain between
    matmuls: measured 630ns/matmul (isolated-cold) vs ~215ns pipelined.
  - softmax has no max-subtraction: |sim| < ~0.5 by construction.
  - exp runs on ACT at FD=1024 ([128,1024] PSUM->SBUF bf16), the intended
    bottleneck engine (~285us/core).
  - The AV matmul's stationary is the V^T block laced with ones columns, so
    output row 0 is the softmax denominator; reciprocal on DVE; broadcast of
    the reciprocal via a [128,128]-padded fp32 matmul whose stationary has a
    single row of ones.
"""

import sys

sys.path.insert(0, "/opt/trn_rl_repo")

import numpy as np
import ml_dtypes

import concourse.bass as bass
import concourse.bacc as bacc
import concourse.tile as tile
from concourse import mybir
from concourse.bass_utils import run_bass_kernel_spmd

BF16 = mybir.dt.bfloat16
F32 = mybir.dt.float32

HEADS = 4
DH = 32  # dim per head
C = 128  # channels
N = 4096  # tokens (64*64)
IC = 512  # i-chunk
NICH = N // IC  # 8
JS = 128  # j-strip
NJS = N // JS  # 32
VBLK = 2 * (DH + 1)  # 66: [1|Vh0|1|Vh1] per j-strip
VTW = VBLK * NJS + 33 + 128  # vt width incl. padding for the 128-wide lhsT AP

_last_results = None  # test harness pokes this for exec_time_ns / profile


def _build():
    nc = bacc.Bacc(None, target_bir_lowering=False)
    xt_d = nc.declare_dram_parameter("xt", [C, N], BF16, isOutput=False)
    wqkT_d = nc.declare_dram_parameter("wqkT", [C, 256], BF16, isOutput=False)
    wvT_d = nc.declare_dram_parameter("wvT", [C, VBLK], BF16, isOutput=False)
    woT_d = nc.declare_dram_parameter("woT", [C, 256], BF16, isOutput=False)
    bo_d = nc.declare_dram_parameter("bo", [C, 1], F32, isOutput=False)
    out_d = nc.declare_dram_parameter("out", [C, N], F32, isOutput=True)

    EXP = mybir.ActivationFunctionType.Exp

    with tile.TileContext(nc) as tc:
        with (
            tc.tile_pool(name="singles", bufs=1) as singles,
            tc.tile_pool(name="pts", bufs=3) as pts,
            tc.tile_pool(name="simpool", bufs=2, space="PSUM") as simpool,
            tc.tile_pool(name="avpool", bufs=2, space="PSUM") as avpool,
            tc.tile_pool(name="pbpool", bufs=1, space="PSUM") as pbpool,
            tc.tile_pool(name="popool", bufs=1, space="PSUM") as popool,
        ):
            xt_s = singles.tile([C, N], BF16)
            wqkT_s = singles.tile([C, 256], BF16)
            wvT_s = singles.tile([C, VBLK], BF16)
            woT_s = singles.tile([C, 256], BF16)
            bo_s = singles.tile([C, 1], F32)
            qk0 = singles.tile([C, 2 * N], BF16)  # head0: rows 0-31 Q|K, 32-127 zero
            qk1 = singles.tile([C, 2 * N], BF16)  # head1
            vt = singles.tile([C, VTW], BF16)
            avu = singles.tile([DH + 1, 2 * N], F32)
            avn = singles.tile([C, 2 * N], BF16)  # rows 33-127 zero
            rec = singles.tile([C, 2 * N], F32)  # row 0 = 1/denom, rows 1+ zero
            ones_s = singles.tile([C, C], F32)  # row 0 ones, rest zero
            outs = singles.tile([C, N], F32)

            nc.sync.dma_start(out=wqkT_s[:], in_=wqkT_d[:])
            nc.sync.dma_start(out=wvT_s[:], in_=wvT_d[:])
            nc.sync.dma_start(out=woT_s[:], in_=woT_d[:])
            nc.sync.dma_start(out=bo_s[:], in_=bo_d[:])
            nc.vector.memset(qk0[:], 0.0)
            nc.vector.memset(qk1[:], 0.0)
            nc.vector.memset(vt[:], 0.0)
            nc.vector.memset(avn[:], 0.0)
            nc.vector.memset(rec[:], 0.0)
            nc.vector.memset(ones_s[:], 0.0)
            nc.vector.memset(ones_s[0:1, :], 1.0)
            for ic in range(NICH):
                nc.sync.dma_start(
                    out=xt_s[:, IC * ic : IC * (ic + 1)],
                    in_=xt_d[:, IC * ic : IC * (ic + 1)],
                )

            # ---- QK projection. Stationary slices of the padded [128,256]
            # weight tile keep M=128 (untiled PE mode); only out rows 0-31 are
            # the target head's projection, copied into that head's qk tile.
            qks = [qk0, qk1]
            for ic in range(NICH):
                for half in range(2):  # 0: Q pair, 1: K pair
                    ps = simpool.tile([128, 1024], F32, tag="sim")
                    for h in range(2):
                        c = 2 * half + h
                        nc.tensor.matmul(
                            ps[:, IC * h : IC * (h + 1)],
                            lhsT=wqkT_s[:, 32 * c : 32 * c + 128],
                            rhs=xt_s[:, IC * ic : IC * (ic + 1)],
                            start=True,
                            stop=True,
                        )
                    for h in range(2):
                        nc.vector.tensor_copy(
                            qks[h][0:DH, N * half + IC * ic : N * half + IC * (ic + 1)],
                            ps[0:DH, IC * h : IC * (h + 1)],
                        )

            # ---- V^T projection into the ones-laced layout.
            # wvT is host-padded to 66 cols with zeros at cols 0 and 33; the
            # projection writes [junk|Vh0|junk|Vh1] to PSUM, DVE memsets the
            # two junk columns to 1.0 (same-engine, no extra semaphore on the
            # copy), then one contiguous aligned copy lands the whole block.
            for jc in range(NJS):
                pv = simpool.tile([128, 1024], F32, tag="sim")
                nc.tensor.matmul(
                    pv[:, 0:VBLK],
                    lhsT=xt_s[:, JS * jc : JS * (jc + 1)],
                    rhs=wvT_s[:],
                    start=True,
                    stop=True,
                )
                nc.vector.memset(pv[:, 0:1], 1.0)
                nc.vector.memset(pv[:, 33:34], 1.0)
                base = VBLK * jc
                nc.vector.tensor_copy(vt[:, base : base + VBLK], pv[:, 0:VBLK])

            # ---- main attention loops (heads sequential)
            for h in range(2):
                qk = qks[h]
                hoff = N * h  # free offset into avu/avn/rec for this head
                for ic in range(NICH):
                    av = avpool.tile([C, IC], F32, tag="av")
                    for g in range(NJS // 2):
                        sim = simpool.tile([128, 1024], F32, tag="sim")
                        for t in range(2):
                            js = 2 * g + t
                            nc.tensor.matmul(
                                sim[:, IC * t : IC * (t + 1)],
                                lhsT=qk[:, N + JS * js : N + JS * (js + 1)],
                                rhs=qk[:, IC * ic : IC * (ic + 1)],
                                start=True,
                                stop=True,
                            )
                        pt = pts.tile([128, 1024], BF16, tag="pt")
                        nc.scalar.activation(pt[:], sim[:], EXP)
                        for t in range(2):
                            js = 2 * g + t
                            nc.tensor.matmul(
                                av[:],
                                lhsT=vt[:, VBLK * js + 33 * h : VBLK * js + 33 * h + 128],
                                rhs=pt[:, IC * t : IC * (t + 1)],
                                start=(g == 0 and t == 0),
                                stop=(g == NJS // 2 - 1 and t == 1),
                            )
                    # ---- per-(head, i-chunk) epilogue
                    sl = slice(hoff + IC * ic, hoff + IC * (ic + 1))
                    nc.vector.tensor_copy(avu[:, sl], av[0 : DH + 1, :])
                    nc.vector.reciprocal(rec[0:1, sl], avu[0:1, sl])
                    pb = pbpool.tile([C, IC], F32, tag="pb")
                    nc.tensor.matmul(
                        pb[:], lhsT=ones_s[:], rhs=rec[:, sl], start=True, stop=True
                    )
                    nc.vector.tensor_mul(
                        avn[0 : DH + 1, sl], avu[:, sl], pb[0 : DH + 1, :]
                    )
                    if h == 1:
                        po = popool.tile([C, IC], F32, tag="po")
                        nc.tensor.matmul(
                            po[:],
                            lhsT=woT_s[:, 0:128],
                            rhs=avn[:, IC * ic : IC * (ic + 1)],
                            start=True,
                            stop=False,
                        )
                        nc.tensor.matmul(
                            po[:],
                            lhsT=woT_s[:, 128:256],
                            rhs=avn[:, N + IC * ic : N + IC * (ic + 1)],
                            start=False,
                            stop=True,
                        )
                        nc.vector.tensor_scalar_add(
                            outs[:, IC * ic : IC * (ic + 1)], po[:], bo_s[:]
                        )
                        nc.sync.dma_start(
                            out=out_d[:, IC * ic : IC * (ic + 1)],
                            in_=outs[:, IC * ic : IC * (ic + 1)],
                        )
    nc.finalize()
    return nc


_nc_cache = None


def _get_nc():
    global _nc_cache
    if _nc_cache is None:
        _nc_cache = _build()
    return _nc_cache


def make_in_maps(x, wq, wk, wv, wo, bo):
    b = 4
    xt = np.asarray(x, np.float32).reshape(b, C, N)
    wq = np.asarray(wq, np.float32)
    wk = np.asarray(wk, np.float32)
    wv = np.asarray(wv, np.float32)
    wo = np.asarray(wo, np.float32)
    bo = np.asarray(bo, np.float32)
    scale = DH ** (-0.5)

    def bf(a):
        return np.ascontiguousarray(a.astype(ml_dtypes.bfloat16))

    in_maps = []
    for core in range(8):
        bi, hp = core // 2, core % 2
        wq2 = wq[64 * hp : 64 * hp + 64] * scale
        wk2 = wk[64 * hp : 64 * hp + 64]
        wv2 = wv[64 * hp : 64 * hp + 64]
        wqkT = np.zeros((C, 256), np.float32)
        wqkT[:, 0:64] = wq2.T  # cols 0-31 Qh0, 32-63 Qh1
        wqkT[:, 64:128] = wk2.T  # cols 64-95 Kh0, 96-127 Kh1
        wvT = np.zeros((C, VBLK), np.float32)  # cols 0,33 stay 0 (psum memset->1)
        wvT[:, 1:33] = wv2.T[:, 0:32]
        wvT[:, 34:66] = wv2.T[:, 32:64]
        woT = np.zeros((C, 256), np.float32)
        woT[1:33, 0:128] = wo[:, 64 * hp : 64 * hp + 32].T
        woT[1:33, 128:256] = wo[:, 64 * hp + 32 : 64 * hp + 64].T
        bo_in = (bo if hp == 0 else np.zeros_like(bo)).reshape(C, 1).astype(np.float32)
        in_maps.append(
            {
                "xt": bf(xt[bi]),
                "wqkT": bf(wqkT),
                "wvT": bf(wvT),
                "woT": bf(woT),
                "bo": bo_in,
            }
        )
    return in_maps


def kernel(x, wq, wk, wv, wo, bo):
    global _last_results
    in_maps = make_in_maps(x, wq, wk, wv, wo, bo)
    nc = _get_nc()
    res = run_bass_kernel_spmd(nc, in_maps, core_ids=list(range(8)))
    _last_results = res
    outs = res.results
    out = np.zeros((4, C, N), np.float32)
    for bi in range(4):
        out[bi] = np.asarray(outs[2 * bi]["out"], np.float32) + np.asarray(
            outs[2 * bi + 1]["out"], np.float32
        )
    return out.reshape(4, C, 64, 64)


# revision 21
# speedup vs baseline: 1.4480x; 1.1275x over previous
"""Trainium2 8-core attention kernel for nn_Attention_14104672600564.

Problem: x[4,128,64,64] f32; wq/wk/wv/wo [128,128]; bo[128].
  per (b,h): sim = (wq x)^T (wk x) * d^-.5 ; attn = softmax(sim) ; out_h = attn @ (wv x)^T
  out = wo @ concat_h(out_h^T) + bo

Sharding: 16 independent (batch, head) attention problems -> 8 cores; each core
gets one batch and one head-pair. Each core computes its partial output
wo[:, headslice] @ heads_out [128, 4096]; the host unshards by summing the two
cores of each batch.

Perf design notes:
  - All matmuls bf16 (1 cyc/row; fp32 would be 4), accumulation fp32 in PSUM.
  - EVERY matmul is padded to untiled (128,128) PE mode (K and M padded to >64
    with zero rows/cols). Mixing PE tile modes forces a drain between
    matmuls: measured 630ns/matmul (isolated-cold) vs ~215ns pipelined.
  - softmax has no max-subtraction: |sim| < ~0.5 by construction.
  - exp runs on ACT at FD=1024 ([128,1024] PSUM->SBUF bf16), the intended
    bottleneck engine (~285us/core).
  - The AV matmul's stationary is the V^T block laced with ones columns, so
    output row 0 is the softmax denominator; reciprocal on DVE; broadcast of
    the reciprocal via a [128,128]-padded fp32 matmul whose stationary has a
    single row of ones.
"""

import sys

sys.path.insert(0, "/opt/trn_rl_repo")

import numpy as np
import ml_dtypes

import concourse.bass as bass
import concourse.bacc as bacc
import concourse.tile as tile
from concourse import mybir
import concourse.bass_utils as _bass_utils
from concourse.bass_utils import run_bass_kernel_spmd



BF16 = mybir.dt.bfloat16
F32 = mybir.dt.float32

HEADS = 4
DH = 32  # dim per head
C = 128  # channels
N = 4096  # tokens (64*64)
IC = 512  # i-chunk
NICH = N // IC  # 8
JS = 128  # j-strip
NJS = N // JS  # 32
VBLK = 2 * (DH + 1)  # 66: [1|Vh0|1|Vh1] per j-strip
VTW = VBLK * NJS + 33 + 128  # vt width incl. padding for the 128-wide lhsT AP

_last_results = None  # test harness pokes this for exec_time_ns / profile


def _build():
    nc = bacc.Bacc(None, target_bir_lowering=False)
    xt_d = nc.declare_dram_parameter("xt", [C, N], BF16, isOutput=False)
    wqkT_d = nc.declare_dram_parameter("wqkT", [C, 512], BF16, isOutput=False)
    wvT_d = nc.declare_dram_parameter("wvT", [C, VBLK], BF16, isOutput=False)
    woT_d = nc.declare_dram_parameter("woT", [C, 256], BF16, isOutput=False)
    bo_d = nc.declare_dram_parameter("bo", [C, 1], F32, isOutput=False)
    out_d = nc.declare_dram_parameter("out", [C, N], F32, isOutput=True)

    EXP = mybir.ActivationFunctionType.Exp

    with tile.TileContext(nc) as tc:
        with (
            tc.tile_pool(name="singles", bufs=1) as singles,
            tc.tile_pool(name="pts", bufs=3) as pts,
            tc.tile_pool(name="simpool", bufs=2, space="PSUM") as simpool,
            tc.tile_pool(name="avpool", bufs=1, space="PSUM") as avpool,
            tc.tile_pool(name="pbpool", bufs=1, space="PSUM") as pbpool,
            tc.tile_pool(name="popool", bufs=1, space="PSUM") as popool,
        ):
            xt_s = singles.tile([C, N], BF16)
            wqkT_s = singles.tile([C, 512], BF16)
            wvT_s = singles.tile([C, VBLK], BF16)
            woT_s = singles.tile([C, 256], BF16)
            bo_s = singles.tile([C, 1], F32)
            qk0 = singles.tile([C, 2 * N], BF16)  # head0: rows 0-31 Q|K, 32-127 zero
            qk1 = singles.tile([C, 2 * N], BF16)  # head1
            vt = singles.tile([C, VTW], BF16)
            avu = singles.tile([DH + 1, 2 * N], F32)
            avn = singles.tile([C, 2 * N], BF16)  # rows 33-127 zero
            rec = singles.tile([1, 2 * N], F32)  # 1/denom (full precision)
            rec_b = singles.tile([C, 2 * N], BF16)  # row 0 = bf16 recips, rows 1+ zero
            ones_s = singles.tile([C, C], BF16)  # row 0 ones, rest zero
            outs = singles.tile([C, N], F32)

            nc.sync.dma_start(out=wqkT_s[:], in_=wqkT_d[:])
            nc.sync.dma_start(out=wvT_s[:], in_=wvT_d[:])
            nc.sync.dma_start(out=woT_s[:], in_=woT_d[:])
            nc.sync.dma_start(out=bo_s[:], in_=bo_d[:])
            nc.vector.memset(vt[:], 0.0)
            nc.vector.memset(avn[:], 0.0)
            nc.vector.memset(rec_b[:], 0.0)
            nc.vector.memset(ones_s[:], 0.0)
            nc.vector.memset(ones_s[0:1, :], 1.0)
            for ic in range(NICH):
                nc.sync.dma_start(
                    out=xt_s[:, IC * ic : IC * (ic + 1)],
                    in_=xt_d[:, IC * ic : IC * (ic + 1)],
                )

            # ---- QK projection. wqkT is host-padded to [128, 512] with each
            # head-tensor's 32 columns at a 128-col stride and zeros elsewhere,
            # so every stationary slice is [W(32)|0(96)]: out rows 32-127 are
            # genuine zeros and the full [128,512] PSUM block lands in the qk
            # tile with no separate zero-fill. Evacuation copies run on ACT
            # (idle during the prologue; DVE was the prologue bottleneck).
            qks = [qk0, qk1]
            for ic in range(NICH):
                for half in range(2):  # 0: Q pair, 1: K pair
                    ps = simpool.tile([128, 1024], F32, tag="sim")
                    for h in range(2):
                        c = 2 * half + h
                        nc.tensor.matmul(
                            ps[:, IC * h : IC * (h + 1)],
                            lhsT=wqkT_s[:, 128 * c : 128 * (c + 1)],
                            rhs=xt_s[:, IC * ic : IC * (ic + 1)],
                            start=True,
                            stop=True,
                        )
                    for h in range(2):
                        nc.scalar.copy(
                            qks[h][:, N * half + IC * ic : N * half + IC * (ic + 1)],
                            ps[:, IC * h : IC * (h + 1)],
                        )

            # ---- V^T projection into the ones-laced layout.
            # wvT is host-padded to 66 cols with zeros at cols 0 and 33; the
            # projection writes [junk|Vh0|junk|Vh1] to PSUM, DVE memsets the
            # two junk columns to 1.0, then one contiguous copy (on ACT) lands
            # the whole block.
            for jc in range(NJS):
                pv = simpool.tile([128, 1024], F32, tag="sim")
                nc.tensor.matmul(
                    pv[:, 0:VBLK],
                    lhsT=xt_s[:, JS * jc : JS * (jc + 1)],
                    rhs=wvT_s[:],
                    start=True,
                    stop=True,
                )
                nc.vector.memset(pv[:, 0:1], 1.0)
                nc.vector.memset(pv[:, 33:34], 1.0)
                base = VBLK * jc
                nc.scalar.copy(vt[:, base : base + VBLK], pv[:, 0:VBLK])

            # ---- main attention loops (heads sequential).
            # Groups are (j-strip, i-chunk-pair): the two sim matmuls share one
            # stationary (the K strip) and the two AV matmuls share another
            # (the vt block), so each LDWEIGHTS serves two matmuls and
            # same-weight matmuls pipeline back-to-back on the PE.
            for h in range(2):
                qk = qks[h]
                hoff = N * h  # free offset into avu/avn/rec for this head
                for ip in range(NICH // 2):
                    ica, icb = 2 * ip, 2 * ip + 1
                    ava = avpool.tile([C, IC], F32, tag="av_a")
                    avb = avpool.tile([C, IC], F32, tag="av_b")
                    for js in range(NJS):
                        sim = simpool.tile([128, 1024], F32, tag="sim")
                        for t, ic in enumerate((ica, icb)):
                            nc.tensor.matmul(
                                sim[:, IC * t : IC * (t + 1)],
                                lhsT=qk[:, N + JS * js : N + JS * (js + 1)],
                                rhs=qk[:, IC * ic : IC * (ic + 1)],
                                start=True,
                                stop=True,
                            )
                        pt = pts.tile([128, 1024], BF16, tag="pt")
                        nc.scalar.activation(pt[:], sim[:], EXP)
                        for t, av in enumerate((ava, avb)):
                            nc.tensor.matmul(
                                av[:],
                                lhsT=vt[:, VBLK * js + 33 * h : VBLK * js + 33 * h + 128],
                                rhs=pt[:, IC * t : IC * (t + 1)],
                                start=(js == 0),
                                stop=(js == NJS - 1),
                            )
                    # ---- per-(head, i-chunk) epilogue
                    for ic, av in ((ica, ava), (icb, avb)):
                        sl = slice(hoff + IC * ic, hoff + IC * (ic + 1))
                        nc.vector.tensor_copy(avu[:, sl], av[0 : DH + 1, :])
                        nc.vector.reciprocal(rec[0:1, sl], avu[0:1, sl])
                        nc.vector.tensor_copy(rec_b[0:1, sl], rec[0:1, sl])
                        pb = pbpool.tile([C, IC], F32, tag="pb")
                        nc.tensor.matmul(
                            pb[:],
                            lhsT=ones_s[:],
                            rhs=rec_b[:, sl],
                            start=True,
                            stop=True,
                        )
                        nc.vector.tensor_mul(
                            avn[0 : DH + 1, sl], avu[:, sl], pb[0 : DH + 1, :]
                        )
                        if h == 1:
                            po = popool.tile([C, IC], F32, tag="po")
                            nc.tensor.matmul(
                                po[:],
                                lhsT=woT_s[:, 0:128],
                                rhs=avn[:, IC * ic : IC * (ic + 1)],
                                start=True,
                                stop=False,
                            )
                            nc.tensor.matmul(
                                po[:],
                                lhsT=woT_s[:, 128:256],
                                rhs=avn[:, N + IC * ic : N + IC * (ic + 1)],
                                start=False,
                                stop=True,
                            )
                            nc.vector.tensor_scalar_add(
                                outs[:, IC * ic : IC * (ic + 1)], po[:], bo_s[:]
                            )
                            nc.sync.dma_start(
                                out=out_d[:, IC * ic : IC * (ic + 1)],
                                in_=outs[:, IC * ic : IC * (ic + 1)],
                            )
    nc.finalize()
    return nc


_nc_cache = None


def _get_nc():
    global _nc_cache
    if _nc_cache is None:
        _nc_cache = _build()
    return _nc_cache


def make_in_maps(x, wq, wk, wv, wo, bo):
    b = 4
    xt = np.asarray(x, np.float32).reshape(b, C, N)
    wq = np.asarray(wq, np.float32)
    wk = np.asarray(wk, np.float32)
    wv = np.asarray(wv, np.float32)
    wo = np.asarray(wo, np.float32)
    bo = np.asarray(bo, np.float32)
    scale = DH ** (-0.5)

    def bf(a):
        return np.ascontiguousarray(a.astype(ml_dtypes.bfloat16))

    in_maps = []
    for core in range(8):
        bi, hp = core // 2, core % 2
        wq2 = wq[64 * hp : 64 * hp + 64] * scale
        wk2 = wk[64 * hp : 64 * hp + 64]
        wv2 = wv[64 * hp : 64 * hp + 64]
        wqkT = np.zeros((C, 512), np.float32)
        wqkT[:, 0:32] = wq2.T[:, 0:32]  # Qh0
        wqkT[:, 128:160] = wq2.T[:, 32:64]  # Qh1
        wqkT[:, 256:288] = wk2.T[:, 0:32]  # Kh0
        wqkT[:, 384:416] = wk2.T[:, 32:64]  # Kh1
        wvT = np.zeros((C, VBLK), np.float32)  # cols 0,33 stay 0 (psum memset->1)
        wvT[:, 1:33] = wv2.T[:, 0:32]
        wvT[:, 34:66] = wv2.T[:, 32:64]
        woT = np.zeros((C, 256), np.float32)
        woT[1:33, 0:128] = wo[:, 64 * hp : 64 * hp + 32].T
        woT[1:33, 128:256] = wo[:, 64 * hp + 32 : 64 * hp + 64].T
        bo_in = (bo if hp == 0 else np.zeros_like(bo)).reshape(C, 1).astype(np.float32)
        in_maps.append(
            {
                "xt": bf(xt[bi]),
                "wqkT": bf(wqkT),
                "wvT": bf(wvT),
                "woT": bf(woT),
                "bo": bo_in,
            }
        )
    return in_maps


def kernel(x, wq, wk, wv, wo, bo):
    global _last_results
    in_maps = make_in_maps(x, wq, wk, wv, wo, bo)
    nc = _get_nc()
    res = run_bass_kernel_spmd(nc, in_maps, core_ids=list(range(8)))
    _last_results = res
    outs = res.results
    out = np.zeros((4, C, N), np.float32)
    for bi in range(4):
        out[bi] = np.asarray(outs[2 * bi]["out"], np.float32) + np.asarray(
            outs[2 * bi + 1]["out"], np.float32
        )
    return out.reshape(4, C, 64, 64)


# revision 39
# speedup vs baseline: 1.6519x; 1.1408x over previous
"""Trainium2 8-core attention kernel for nn_Attention_14104672600564.

Problem: x[4,128,64,64] f32; wq/wk/wv/wo [128,128]; bo[128].
  per (b,h): sim = (wq x)^T (wk x) * d^-.5 ; attn = softmax(sim) ; out_h = attn @ (wv x)^T
  out = wo @ concat_h(out_h^T) + bo

Sharding: 16 independent (batch, head) attention problems -> 8 cores; each core
gets one batch and one head-pair. Each core computes its partial output
wo[:, headslice] @ heads_out [128, 4096]; the host unshards by summing the two
cores of each batch.

Perf design notes:
  - All matmuls bf16 (1 cyc/row; fp32 would be 4), accumulation fp32 in PSUM.
  - EVERY matmul is padded to untiled (128,128) PE mode (K and M padded to >64
    with zero rows/cols). Mixing PE tile modes forces a drain between
    matmuls: measured 630ns/matmul (isolated-cold) vs ~215ns pipelined.
  - softmax has no max-subtraction: |sim| < ~0.5 by construction.
  - exp runs on ACT at FD=1024 ([128,1024] PSUM->SBUF bf16), the intended
    bottleneck engine (~285us/core).
  - The AV matmul's stationary is the V^T block laced with ones columns, so
    output row 0 is the softmax denominator; reciprocal on DVE; broadcast of
    the reciprocal via a [128,128]-padded fp32 matmul whose stationary has a
    single row of ones.
"""

import sys

sys.path.insert(0, "/opt/trn_rl_repo")

import numpy as np
import ml_dtypes

import concourse.bass as bass
import concourse.bacc as bacc
import concourse.tile as tile
from concourse import mybir
import concourse.bass_utils as _bass_utils
from concourse.bass_utils import run_bass_kernel_spmd



BF16 = mybir.dt.bfloat16
F32 = mybir.dt.float32

HEADS = 4
DH = 32  # dim per head
C = 128  # channels
N = 4096  # tokens (64*64)
IC = 512  # i-chunk
NICH = N // IC  # 8
JS = 128  # j-strip
NJS = N // JS  # 32
VBLK = 2 * (DH + 1)  # 66: [1|Vh0|1|Vh1] per j-strip
VTW = VBLK * NJS + 33 + 128  # vt width incl. padding for the 128-wide lhsT AP

_last_results = None  # test harness pokes this for exec_time_ns / profile


def _build():
    nc = bacc.Bacc(None, target_bir_lowering=False)
    xt_d = nc.declare_dram_parameter("xt", [C, N], BF16, isOutput=False)
    wqkT_d = nc.declare_dram_parameter("wqkT", [C, 512], BF16, isOutput=False)
    wvT_d = nc.declare_dram_parameter("wvT", [C, VBLK], BF16, isOutput=False)
    woT_d = nc.declare_dram_parameter("woT", [C, 256], BF16, isOutput=False)
    bo_d = nc.declare_dram_parameter("bo", [C, 1], F32, isOutput=False)
    out_d = nc.declare_dram_parameter("out", [C, N], F32, isOutput=True)

    EXP = mybir.ActivationFunctionType.Exp

    with tile.TileContext(nc) as tc:
        with (
            tc.tile_pool(name="singles", bufs=1) as singles,
            tc.tile_pool(name="pts", bufs=4) as pts,
            tc.tile_pool(name="simpool", bufs=2, space="PSUM") as simpool,
            tc.tile_pool(name="avpool", bufs=1, space="PSUM") as avpool,
            tc.tile_pool(name="pbpool", bufs=1, space="PSUM") as pbpool,
            tc.tile_pool(name="popool", bufs=1, space="PSUM") as popool,
        ):
            xt_s = singles.tile([C, N], BF16)
            wqkT_s = singles.tile([C, 512], BF16)
            wvT_s = singles.tile([C, VBLK], BF16)
            woT_s = singles.tile([C, 256], BF16)
            bo_s = singles.tile([C, 1], F32)
            qk0 = singles.tile([C, 2 * N], BF16)  # head0: rows 0-31 Q|K, 32-127 zero
            qk1 = singles.tile([C, 2 * N], BF16)  # head1
            vt = singles.tile([C, VTW], BF16)
            avu = singles.tile([DH + 1, 2 * N], F32)
            avn = singles.tile([C, 2 * N], BF16)  # rows 33-127 zero
            rec = singles.tile([1, 2 * N], F32)  # 1/denom (full precision)
            rscr = singles.tile([1, IC], F32)  # reciprocal NR scratch
            rec_b = singles.tile([C, 2 * N], BF16)  # row 0 = bf16 recips, rows 1+ zero
            ones_s = singles.tile([C, C], BF16)  # row 0 ones, rest zero
            outs = singles.tile([C, N], F32)

            nc.sync.dma_start(out=wqkT_s[:], in_=wqkT_d[:])
            nc.sync.dma_start(out=wvT_s[:], in_=wvT_d[:])
            nc.sync.dma_start(out=woT_s[:], in_=woT_d[:])
            nc.sync.dma_start(out=bo_s[:], in_=bo_d[:])
            nc.vector.memset(vt[:], 0.0)
            nc.vector.memset(avn[:], 0.0)
            nc.vector.memset(rec_b[:], 0.0)
            nc.vector.memset(ones_s[:], 0.0)
            nc.vector.memset(ones_s[0:1, :], 1.0)
            for ic in range(NICH):
                nc.sync.dma_start(
                    out=xt_s[:, IC * ic : IC * (ic + 1)],
                    in_=xt_d[:, IC * ic : IC * (ic + 1)],
                )

            # ---- QK projection. wqkT is host-padded to [128, 512] with each
            # head-tensor's 32 columns at a 128-col stride and zeros elsewhere,
            # so every stationary slice is [W(32)|0(96)]: out rows 32-127 are
            # genuine zeros and the full [128,512] PSUM block lands in the qk
            # tile with no separate zero-fill. Evacuation copies run on ACT
            # (idle during the prologue; DVE was the prologue bottleneck).
            qks = [qk0, qk1]
            for ic in range(NICH):
                for half in range(2):  # 0: Q pair, 1: K pair
                    ps = simpool.tile([128, 1024], F32, tag="sim")
                    for h in range(2):
                        c = 2 * half + h
                        nc.tensor.matmul(
                            ps[:, IC * h : IC * (h + 1)],
                            lhsT=wqkT_s[:, 128 * c : 128 * (c + 1)],
                            rhs=xt_s[:, IC * ic : IC * (ic + 1)],
                            start=True,
                            stop=True,
                        )
                    for h in range(2):
                        nc.scalar.copy(
                            qks[h][:, N * half + IC * ic : N * half + IC * (ic + 1)],
                            ps[:, IC * h : IC * (h + 1)],
                        )

            # ---- V^T projection into the ones-laced layout.
            # wvT is host-padded to 66 cols with zeros at cols 0 and 33; the
            # projection writes [junk|Vh0|junk|Vh1] to PSUM, DVE memsets the
            # two junk columns to 1.0, then one contiguous copy (on ACT) lands
            # the whole block.
            for jc in range(NJS):
                pv = simpool.tile([128, 1024], F32, tag="sim")
                nc.tensor.matmul(
                    pv[:, 0:VBLK],
                    lhsT=xt_s[:, JS * jc : JS * (jc + 1)],
                    rhs=wvT_s[:],
                    start=True,
                    stop=True,
                )
                nc.vector.memset(pv[:, 0:1], 1.0)
                nc.vector.memset(pv[:, 33:34], 1.0)
                base = VBLK * jc
                nc.scalar.copy(vt[:, base : base + VBLK], pv[:, 0:VBLK])

            # ---- main attention loops (heads sequential).
            # Groups are (j-strip, i-chunk-pair): the two sim matmuls share one
            # stationary (the K strip) and the two AV matmuls share another
            # (the vt block), so each LDWEIGHTS serves two matmuls and
            # same-weight matmuls pipeline back-to-back on the PE.
            for h in range(2):
                qk = qks[h]
                hoff = N * h  # free offset into avu/avn/rec for this head
                for ip in range(NICH // 2):
                    ica, icb = 2 * ip, 2 * ip + 1
                    ava = avpool.tile([C, IC], F32, tag="av_a")
                    avb = avpool.tile([C, IC], F32, tag="av_b")
                    for js in range(NJS):
                        sim = simpool.tile([128, 1024], F32, tag="sim")
                        for t, ic in enumerate((ica, icb)):
                            nc.tensor.matmul(
                                sim[:, IC * t : IC * (t + 1)],
                                lhsT=qk[:, N + JS * js : N + JS * (js + 1)],
                                rhs=qk[:, IC * ic : IC * (ic + 1)],
                                start=True,
                                stop=True,
                            )
                        pt = pts.tile([128, 1024], BF16, tag="pt")
                        nc.scalar.activation(pt[:], sim[:], EXP)
                        for t, av in enumerate((ava, avb)):
                            nc.tensor.matmul(
                                av[:],
                                lhsT=vt[:, VBLK * js + 33 * h : VBLK * js + 33 * h + 128],
                                rhs=pt[:, IC * t : IC * (t + 1)],
                                start=(js == 0),
                                stop=(js == NJS - 1),
                            )
                    # ---- per-(head, i-chunk) epilogue
                    for ic, av in ((ica, ava), (icb, avb)):
                        sl = slice(hoff + IC * ic, hoff + IC * (ic + 1))
                        nc.vector.tensor_copy(avu[:, sl], av[0 : DH + 1, :])
                        nc.vector.reciprocal_approx_accurate(
                            rec[0:1, sl], avu[0:1, sl], scratch=rscr[0:1, :]
                        )
                        nc.vector.tensor_copy(rec_b[0:1, sl], rec[0:1, sl])
                        pb = pbpool.tile([C, IC], F32, tag="pb")
                        nc.tensor.matmul(
                            pb[:],
                            lhsT=ones_s[:],
                            rhs=rec_b[:, sl],
                            start=True,
                            stop=True,
                        )
                        nc.vector.tensor_mul(
                            avn[0 : DH + 1, sl], avu[:, sl], pb[0 : DH + 1, :]
                        )
                        if h == 1:
                            po = popool.tile([C, IC], F32, tag="po")
                            nc.tensor.matmul(
                                po[:],
                                lhsT=woT_s[:, 0:128],
                                rhs=avn[:, IC * ic : IC * (ic + 1)],
                                start=True,
                                stop=False,
                            )
                            nc.tensor.matmul(
                                po[:],
                                lhsT=woT_s[:, 128:256],
                                rhs=avn[:, N + IC * ic : N + IC * (ic + 1)],
                                start=False,
                                stop=True,
                            )
                            nc.vector.tensor_scalar_add(
                                outs[:, IC * ic : IC * (ic + 1)], po[:], bo_s[:]
                            )
                            nc.sync.dma_start(
                                out=out_d[:, IC * ic : IC * (ic + 1)],
                                in_=outs[:, IC * ic : IC * (ic + 1)],
                            )
    nc.finalize()
    return nc


_nc_cache = None


def _get_nc():
    global _nc_cache
    if _nc_cache is None:
        _nc_cache = _build()
    return _nc_cache


def make_in_maps(x, wq, wk, wv, wo, bo):
    b = 4
    xt = np.asarray(x, np.float32).reshape(b, C, N)
    wq = np.asarray(wq, np.float32)
    wk = np.asarray(wk, np.float32)
    wv = np.asarray(wv, np.float32)
    wo = np.asarray(wo, np.float32)
    bo = np.asarray(bo, np.float32)
    scale = DH ** (-0.5)

    def bf(a):
        return np.ascontiguousarray(a.astype(ml_dtypes.bfloat16))

    in_maps = []
    for core in range(8):
        bi, hp = core // 2, core % 2
        wq2 = wq[64 * hp : 64 * hp + 64] * scale
        wk2 = wk[64 * hp : 64 * hp + 64]
        wv2 = wv[64 * hp : 64 * hp + 64]
        wqkT = np.zeros((C, 512), np.float32)
        wqkT[:, 0:32] = wq2.T[:, 0:32]  # Qh0
        wqkT[:, 128:160] = wq2.T[:, 32:64]  # Qh1
        wqkT[:, 256:288] = wk2.T[:, 0:32]  # Kh0
        wqkT[:, 384:416] = wk2.T[:, 32:64]  # Kh1
        wvT = np.zeros((C, VBLK), np.float32)  # cols 0,33 stay 0 (psum memset->1)
        wvT[:, 1:33] = wv2.T[:, 0:32]
        wvT[:, 34:66] = wv2.T[:, 32:64]
        woT = np.zeros((C, 256), np.float32)
        woT[1:33, 0:128] = wo[:, 64 * hp : 64 * hp + 32].T
        woT[1:33, 128:256] = wo[:, 64 * hp + 32 : 64 * hp + 64].T
        bo_in = (bo if hp == 0 else np.zeros_like(bo)).reshape(C, 1).astype(np.float32)
        in_maps.append(
            {
                "xt": bf(xt[bi]),
                "wqkT": bf(wqkT),
                "wvT": bf(wvT),
                "woT": bf(woT),
                "bo": bo_in,
            }
        )
    return in_maps


def kernel(x, wq, wk, wv, wo, bo):
    global _last_results
    in_maps = make_in_maps(x, wq, wk, wv, wo, bo)
    nc = _get_nc()
    res = run_bass_kernel_spmd(nc, in_maps, core_ids=list(range(8)))
    _last_results = res
    outs = res.results
    out = np.zeros((4, C, N), np.float32)
    for bi in range(4):
        out[bi] = np.asarray(outs[2 * bi]["out"], np.float32) + np.asarray(
            outs[2 * bi + 1]["out"], np.float32
        )
    return out.reshape(4, C, 64, 64)


# revision 46
# speedup vs baseline: 1.6579x; 1.0036x over previous
"""Trainium2 8-core attention kernel for nn_Attention_14104672600564.

Problem: x[4,128,64,64] f32; wq/wk/wv/wo [128,128]; bo[128].
  per (b,h): sim = (wq x)^T (wk x) * d^-.5 ; attn = softmax(sim) ; out_h = attn @ (wv x)^T
  out = wo @ concat_h(out_h^T) + bo

Sharding: 16 independent (batch, head) attention problems -> 8 cores; each core
gets one batch and one head-pair. Each core computes its partial output
wo[:, headslice] @ heads_out [128, 4096]; the host unshards by summing the two
cores of each batch.

Perf design notes:
  - All matmuls bf16 (1 cyc/row; fp32 would be 4), accumulation fp32 in PSUM.
  - EVERY matmul is padded to untiled (128,128) PE mode (K and M padded to >64
    with zero rows/cols). Mixing PE tile modes forces a drain between
    matmuls: measured 630ns/matmul (isolated-cold) vs ~215ns pipelined.
  - softmax has no max-subtraction: |sim| < ~0.5 by construction.
  - exp runs on ACT at FD=1024 ([128,1024] PSUM->SBUF bf16), the intended
    bottleneck engine (~285us/core).
  - The AV matmul's stationary is the V^T block laced with ones columns, so
    output row 0 is the softmax denominator; reciprocal on DVE; broadcast of
    the reciprocal via a [128,128]-padded fp32 matmul whose stationary has a
    single row of ones.
"""

import sys

sys.path.insert(0, "/opt/trn_rl_repo")

import numpy as np
import ml_dtypes

import concourse.bass as bass
import concourse.bacc as bacc
import concourse.tile as tile
from concourse import mybir
import concourse.bass_utils as _bass_utils
from concourse.bass_utils import run_bass_kernel_spmd



BF16 = mybir.dt.bfloat16
F32 = mybir.dt.float32

HEADS = 4
DH = 32  # dim per head
C = 128  # channels
N = 4096  # tokens (64*64)
IC = 512  # i-chunk
NICH = N // IC  # 8
JS = 128  # j-strip
NJS = N // JS  # 32
VBLK = 2 * (DH + 1)  # 66: [1|Vh0|1|Vh1] per j-strip
VTW = VBLK * NJS + 33 + 128  # vt width incl. padding for the 128-wide lhsT AP

_last_results = None  # test harness pokes this for exec_time_ns / profile


def _build():
    nc = bacc.Bacc(None, target_bir_lowering=False)
    xt_d = nc.declare_dram_parameter("xt", [C, N], BF16, isOutput=False)
    wqkT_d = nc.declare_dram_parameter("wqkT", [C, 512], BF16, isOutput=False)
    wvT_d = nc.declare_dram_parameter("wvT", [C, VBLK], BF16, isOutput=False)
    woT_d = nc.declare_dram_parameter("woT", [C, 256], BF16, isOutput=False)
    out_d = nc.declare_dram_parameter("out", [C, N], F32, isOutput=True)

    EXP = mybir.ActivationFunctionType.Exp

    with tile.TileContext(nc) as tc:
        with (
            tc.tile_pool(name="singles", bufs=1) as singles,
            tc.tile_pool(name="pts", bufs=4) as pts,
            tc.tile_pool(name="simpool", bufs=2, space="PSUM") as simpool,
            tc.tile_pool(name="avpool", bufs=1, space="PSUM") as avpool,
            tc.tile_pool(name="pbpool", bufs=1, space="PSUM") as pbpool,
            tc.tile_pool(name="popool", bufs=1, space="PSUM") as popool,
        ):
            xt_s = singles.tile([C, N], BF16)
            wqkT_s = singles.tile([C, 512], BF16)
            wvT_s = singles.tile([C, VBLK], BF16)
            woT_s = singles.tile([C, 256], BF16)
            qk0 = singles.tile([C, 2 * N], BF16)  # head0: rows 0-31 Q|K, 32-127 zero
            qk1 = singles.tile([C, 2 * N], BF16)  # head1
            vt = singles.tile([C, VTW], BF16)
            avu = singles.tile([DH + 1, 2 * N], F32)
            avn = singles.tile([C, 2 * N], BF16)  # rows 33-127 zero
            rec = singles.tile([1, 2 * N], F32)  # 1/denom (full precision)
            rscr = singles.tile([1, IC], F32)  # reciprocal NR scratch
            rec_b = singles.tile([C, 2 * N], BF16)  # row 0 = bf16 recips, rows 1+ zero
            ones_s = singles.tile([C, C], BF16)  # row 0 ones, rest zero
            outs = singles.tile([C, N], F32)

            nc.sync.dma_start(out=wqkT_s[:], in_=wqkT_d[:])
            nc.sync.dma_start(out=wvT_s[:], in_=wvT_d[:])
            nc.sync.dma_start(out=woT_s[:], in_=woT_d[:])
            nc.vector.memset(vt[:], 0.0)
            nc.vector.memset(avn[:], 0.0)
            nc.vector.memset(rec_b[:], 0.0)
            nc.vector.memset(ones_s[:], 0.0)
            nc.vector.memset(ones_s[0:1, :], 1.0)
            for ic in range(NICH):
                nc.sync.dma_start(
                    out=xt_s[:, IC * ic : IC * (ic + 1)],
                    in_=xt_d[:, IC * ic : IC * (ic + 1)],
                )

            # ---- QK projection. wqkT is host-padded to [128, 512] with each
            # head-tensor's 32 columns at a 128-col stride and zeros elsewhere,
            # so every stationary slice is [W(32)|0(96)]: out rows 32-127 are
            # genuine zeros and the full [128,512] PSUM block lands in the qk
            # tile with no separate zero-fill. Evacuation copies run on ACT
            # (idle during the prologue; DVE was the prologue bottleneck).
            # Head0's copies go to ACT (fast prologue, exp stream starts right
            # after); head1's go to DVE and drain under head0's main loop.
            qks = [qk0, qk1]
            for h in range(2):
                for ic in range(NICH):
                    ps = simpool.tile([128, 1024], F32, tag="sim")
                    for half in range(2):  # 0: Q, 1: K
                        c = 2 * half + h
                        nc.tensor.matmul(
                            ps[:, IC * half : IC * (half + 1)],
                            lhsT=wqkT_s[:, 128 * c : 128 * (c + 1)],
                            rhs=xt_s[:, IC * ic : IC * (ic + 1)],
                            start=True,
                            stop=True,
                        )
                    for half in range(2):
                        if h == 0:
                            nc.scalar.copy(
                                qks[h][
                                    :, N * half + IC * ic : N * half + IC * (ic + 1)
                                ],
                                ps[:, IC * half : IC * (half + 1)],
                            )
                        else:
                            nc.vector.tensor_copy(
                                qks[h][
                                    :, N * half + IC * ic : N * half + IC * (ic + 1)
                                ],
                                ps[:, IC * half : IC * (half + 1)],
                            )

            # ---- V^T projection into the ones-laced layout.
            # wvT is host-padded to 66 cols with zeros at cols 0 and 33; the
            # projection writes [junk|Vh0|junk|Vh1] to PSUM, DVE memsets the
            # two junk columns to 1.0, then one contiguous copy (on ACT) lands
            # the whole block.
            for jc in range(NJS):
                pv = simpool.tile([128, 1024], F32, tag="sim")
                nc.tensor.matmul(
                    pv[:, 0:VBLK],
                    lhsT=xt_s[:, JS * jc : JS * (jc + 1)],
                    rhs=wvT_s[:],
                    start=True,
                    stop=True,
                )
                nc.vector.memset(pv[:, 0:1], 1.0)
                nc.vector.memset(pv[:, 33:34], 1.0)
                base = VBLK * jc
                nc.scalar.copy(vt[:, base : base + VBLK], pv[:, 0:VBLK])

            # ---- main attention loops (heads sequential).
            # Groups are (j-strip, i-chunk-pair): the two sim matmuls share one
            # stationary (the K strip) and the two AV matmuls share another
            # (the vt block), so each LDWEIGHTS serves two matmuls and
            # same-weight matmuls pipeline back-to-back on the PE.
            for h in range(2):
                qk = qks[h]
                hoff = N * h  # free offset into avu/avn/rec for this head
                for ip in range(NICH // 2):
                    ica, icb = 2 * ip, 2 * ip + 1
                    ava = avpool.tile([C, IC], F32, tag="av_a")
                    avb = avpool.tile([C, IC], F32, tag="av_b")
                    for js in range(NJS):
                        sim = simpool.tile([128, 1024], F32, tag="sim")
                        for t, ic in enumerate((ica, icb)):
                            nc.tensor.matmul(
                                sim[:, IC * t : IC * (t + 1)],
                                lhsT=qk[:, N + JS * js : N + JS * (js + 1)],
                                rhs=qk[:, IC * ic : IC * (ic + 1)],
                                start=True,
                                stop=True,
                            )
                        pt = pts.tile([128, 1024], BF16, tag="pt")
                        nc.scalar.activation(pt[:], sim[:], EXP)
                        for t, av in enumerate((ava, avb)):
                            nc.tensor.matmul(
                                av[:],
                                lhsT=vt[:, VBLK * js + 33 * h : VBLK * js + 33 * h + 128],
                                rhs=pt[:, IC * t : IC * (t + 1)],
                                start=(js == 0),
                                stop=(js == NJS - 1),
                            )
                    # ---- per-(head, i-chunk) epilogue
                    for ic, av in ((ica, ava), (icb, avb)):
                        sl = slice(hoff + IC * ic, hoff + IC * (ic + 1))
                        nc.vector.tensor_copy(avu[:, sl], av[0 : DH + 1, :])
                        nc.vector.reciprocal_approx_accurate(
                            rec[0:1, sl], avu[0:1, sl], scratch=rscr[0:1, :]
                        )
                        nc.vector.tensor_copy(rec_b[0:1, sl], rec[0:1, sl])
                        pb = pbpool.tile([C, IC], F32, tag="pb")
                        nc.tensor.matmul(
                            pb[:],
                            lhsT=ones_s[:],
                            rhs=rec_b[:, sl],
                            start=True,
                            stop=True,
                        )
                        nc.vector.tensor_mul(
                            avn[0 : DH + 1, sl], avu[:, sl], pb[0 : DH + 1, :]
                        )
                        if h == 1:
                            po = popool.tile([C, IC], F32, tag="po")
                            nc.tensor.matmul(
                                po[:],
                                lhsT=woT_s[:, 0:128],
                                rhs=avn[:, IC * ic : IC * (ic + 1)],
                                start=True,
                                stop=False,
                            )
                            nc.tensor.matmul(
                                po[:],
                                lhsT=woT_s[:, 128:256],
                                rhs=avn[:, N + IC * ic : N + IC * (ic + 1)],
                                start=False,
                                stop=True,
                            )
                            # bias is folded into the projection (avn row 33
                            # is all-ones, woT row 33 of block 0 is bo)
                            nc.vector.tensor_copy(
                                outs[:, IC * ic : IC * (ic + 1)], po[:]
                            )
                            nc.sync.dma_start(
                                out=out_d[:, IC * ic : IC * (ic + 1)],
                                in_=outs[:, IC * ic : IC * (ic + 1)],
                            )
    nc.finalize()
    return nc


_nc_cache = None


def _get_nc():
    global _nc_cache
    if _nc_cache is None:
        _nc_cache = _build()
    return _nc_cache


def make_in_maps(x, wq, wk, wv, wo, bo):
    b = 4
    xt = np.asarray(x, np.float32).reshape(b, C, N)
    wq = np.asarray(wq, np.float32)
    wk = np.asarray(wk, np.float32)
    wv = np.asarray(wv, np.float32)
    wo = np.asarray(wo, np.float32)
    bo = np.asarray(bo, np.float32)
    scale = DH ** (-0.5)

    def bf(a):
        return np.ascontiguousarray(a.astype(ml_dtypes.bfloat16))

    in_maps = []
    for core in range(8):
        bi, hp = core // 2, core % 2
        wq2 = wq[64 * hp : 64 * hp + 64] * scale
        wk2 = wk[64 * hp : 64 * hp + 64]
        wv2 = wv[64 * hp : 64 * hp + 64]
        wqkT = np.zeros((C, 512), np.float32)
        wqkT[:, 0:32] = wq2.T[:, 0:32]  # Qh0
        wqkT[:, 128:160] = wq2.T[:, 32:64]  # Qh1
        wqkT[:, 256:288] = wk2.T[:, 0:32]  # Kh0
        wqkT[:, 384:416] = wk2.T[:, 32:64]  # Kh1
        wvT = np.zeros((C, VBLK), np.float32)  # cols 0,33 stay 0 (psum memset->1)
        wvT[:, 1:33] = wv2.T[:, 0:32]
        wvT[:, 34:66] = wv2.T[:, 32:64]
        woT = np.zeros((C, 256), np.float32)
        woT[1:33, 0:128] = wo[:, 64 * hp : 64 * hp + 32].T
        woT[1:33, 128:256] = wo[:, 64 * hp + 32 : 64 * hp + 64].T
        if hp == 0:
            woT[0, 0:128] = bo  # bias rides avn row 0 (= denom/denom = 1)
        in_maps.append(
            {
                "xt": bf(xt[bi]),
                "wqkT": bf(wqkT),
                "wvT": bf(wvT),
                "woT": bf(woT),
            }
        )
    return in_maps


def kernel(x, wq, wk, wv, wo, bo):
    global _last_results
    in_maps = make_in_maps(x, wq, wk, wv, wo, bo)
    nc = _get_nc()
    res = run_bass_kernel_spmd(nc, in_maps, core_ids=list(range(8)))
    _last_results = res
    outs = res.results
    out = np.zeros((4, C, N), np.float32)
    for bi in range(4):
        out[bi] = np.asarray(outs[2 * bi]["out"], np.float32) + np.asarray(
            outs[2 * bi + 1]["out"], np.float32
        )
    return out.reshape(4, C, 64, 64)
